# revision 1
# baseline (speedup 1.0000x reference)
"""BIDAF attention-flow kernel for Trainium2 (Bass/Tile), 8-core data-parallel.

Reference computation (per batch b):
    S[t,j]  = H[t]·w_h + U[j]·w_u + sum_d H[t,d]*U[j,d]*w_hu[d]
    A       = softmax_j(S);          C2Q = A @ U
    b_att   = softmax_t(max_j S);    Q2C = b_att @ H   (broadcast over t)
    G       = [H, C2Q, H*C2Q, H*Q2C]        # [T, 4D]

Kernel strategy (per core, 8 batches):
  * S is computed TRANSPOSED (ST[j,t]) so that the softmax-attention matmul
    (C2Q) can consume P=exp(ST) directly as lhsT with no A-transpose, and the
    moving dim is T (N=512 chunks -> float32r runs at full PE rate).
  * sh[t]=H·w_h cancels inside softmax_j, so P = exp(shu + su[j]) with su as a
    per-partition ACT bias.  sh re-enters only in the tiny [128,8] b_att
    weights: wq = max_j(P) * exp(sh).
  * A ones-column appended to H and U host-side (column 256 of the 260-wide
    inputs) gives l[t]=sum_j P[j,t] and Wsum=sum_t wq[t] for free inside the
    C2Q/Q2C matmuls; the normalizers fold into mandatory PSUM->SBUF copies.
  * max_j P needs a partition reduce: PE re-transposes P ([128,128] blocks
    into PSUM) and one DVE reduce_max does all 8 chunks.
  * identity / ones constants are supplied as extra kernel inputs; partition
    broadcasts are K=1 ones-matmuls.
  * Tile emits multi-wait instructions; TRN2 allows 1 wait/instruction, so
    the bacc rust passes (move_matmul_waits_to_ldweights +
    generate_event_semaphores) are run on the traced module before compile.
"""

import os
import sys

sys.path.insert(0, "/opt/trn_rl_repo")

import numpy as np

import concourse.bass as bass
import concourse.mybir as mybir
from concourse import tile

B, T, J, D = 64, 1024, 128, 256
NCORES = 8
BPC = B // NCORES  # batches per core
P = 128
NT = T // P  # 8 t-chunks per batch
DA = 260  # augmented feature dim: [x | 1 | pad(1.0)*3]
F32 = mybir.dt.float32
F32R = mybir.dt.float32r
AF = mybir.ActivationFunctionType
ALU = mybir.AluOpType
AX = mybir.AxisListType

# float32r streams fp32 bits through the PE at 1 cycle/row for N>=256
# (vs 4 cycles/row for plain float32).
MMDT = F32R


PHASE = int(os.environ.get("KPHASE", "10"))


def build_kernel(nc, bpc):
    H = nc.declare_dram_parameter("H", [bpc, T, DA], F32, isOutput=False)
    U = nc.declare_dram_parameter("U", [bpc, J, DA], F32, isOutput=False)
    whT_in = nc.declare_dram_parameter("whT", [P, 2, 2], F32, isOutput=False)
    wuu = nc.declare_dram_parameter("w2", [2, D], F32, isOutput=False)
    ident_in = nc.declare_dram_parameter("ident", [P, P], F32, isOutput=False)
    ones_in = nc.declare_dram_parameter("ones1", [1, P], F32, isOutput=False)
    G = nc.declare_dram_parameter("G", [bpc, T, 4 * D], F32, isOutput=True)

    with tile.TileContext(nc) as tc:
        with (
            tc.tile_pool(name="const", bufs=1) as const_pool,
            tc.tile_pool(name="h", bufs=2) as h_pool,
            tc.tile_pool(name="ht", bufs=2) as ht_pool,
            tc.tile_pool(name="p", bufs=2) as p_pool,
            tc.tile_pool(name="g", bufs=2) as g_pool,
            tc.tile_pool(name="u", bufs=2) as u_pool,
            tc.tile_pool(name="sm", bufs=2) as sm_pool,
            tc.tile_pool(name="bigps", bufs=2, space="PSUM") as big_ps,
            tc.tile_pool(name="halfps", bufs=2, space="PSUM") as half_ps,
            tc.tile_pool(name="cqps", bufs=1, space="PSUM") as cq_ps,
            tc.tile_pool(name="smps", bufs=1, space="PSUM") as sm_ps,
        ):
            # ---- constants ----
            ident = const_pool.tile([P, P], MMDT)
            nc.sync.dma_start(ident[:], ident_in[:].bitcast(MMDT))
            ones1 = const_pool.tile([1, P], MMDT)
            nc.sync.dma_start(ones1[:], ones_in[:].bitcast(MMDT))
            # w_h in partition-major layout [p, kc, dup] (host-prepared)
            whT = const_pool.tile([P, 2, 2], MMDT)
            nc.sync.dma_start(whT[:], whT_in[:].bitcast(MMDT))
            # broadcast [w_hu; w_u] across partitions via a K=1 ones-matmul
            w2_sb = const_pool.tile([1, 2 * D], MMDT)
            nc.sync.dma_start(
                w2_sb[:],
                wuu[:].rearrange("a d -> (a d)").unsqueeze(0).bitcast(MMDT),
            )
            wps = big_ps.tile([P, 2 * D], F32, tag="big")
            nc.tensor.matmul(wps[:], ones1[:], w2_sb[:], start=True, stop=True)
            wb = const_pool.tile([P, 2 * D], F32)
            nc.scalar.copy(wb[:], wps[:])
            whu_b = wb[:, 0:D]
            wu_b = wb[:, D : 2 * D]

            for b in range(bpc):
                # ---- load inputs (pre-augmented with ones column) ----
                Hn = h_pool.tile([P, NT, DA], F32)
                nc.sync.dma_start(
                    Hn[:], H[b].rearrange("(c p) d -> p c d", p=P)
                )
                Hnr = h_pool.tile([P, NT, DA], MMDT)
                nc.sync.dma_start(Hnr[:], Hn[:].bitcast(MMDT))
                Uo = u_pool.tile([P, DA], MMDT)
                nc.sync.dma_start(Uo[:], U[b].bitcast(MMDT))

                # G block 0 = H (write out as soon as it is on chip)
                Gb = G[b].rearrange("(c p) (g d) -> p c g d", p=P, d=D)
                nc.sync.dma_start(Gb[:, :, 0, :], Hn[:, :, 0:D])

                if PHASE < 2:
                    continue
                # ---- U-side prep ----
                Uw = u_pool.tile([P, D], MMDT)
                nc.vector.tensor_mul(Uw[:], Uo[:, 0:D], whu_b.bitcast(MMDT))
                su = sm_pool.tile([P, 1], F32)
                scr = sm_pool.tile([P, D], F32)
                nc.vector.tensor_mul(scr[:], Uo[:, 0:D].bitcast(F32), wu_b)
                nc.vector.reduce_sum(su[:], scr[:], axis=AX.X)
                uwt_ps = sm_ps.tile([P, 2, P], MMDT, tag="sm")
                for kc in range(2):
                    nc.tensor.transpose(
                        uwt_ps[:, kc, :], Uw[:, kc * P : (kc + 1) * P], ident[:]
                    )
                UwT = u_pool.tile([P, 2, P], MMDT)
                nc.scalar.copy(UwT[:], uwt_ps[:])

                if PHASE < 3:
                    continue
                # ---- H transpose + similarity matmul, in two T-halves ----
                HT = ht_pool.tile([P, 2, T], MMDT)
                st = big_ps.tile([P, T], F32, tag="big")
                for th in range(2):
                    for kc in range(2):
                        htp = half_ps.tile([P, 512], MMDT, tag="half")
                        for i in range(4):
                            c = th * 4 + i
                            nc.tensor.transpose(
                                htp[:, i * P : (i + 1) * P],
                                Hnr[:, c, kc * P : (kc + 1) * P],
                                ident[:],
                            )
                        dst = HT[:, kc, th * 512 : (th + 1) * 512]
                        if kc == 0:
                            nc.scalar.copy(dst, htp[:])
                        else:
                            nc.vector.tensor_copy(dst, htp[:])
                    for kc in range(2):
                        nc.tensor.matmul(
                            st[:, th * 512 : (th + 1) * 512],
                            UwT[:, kc, :],
                            HT[:, kc, th * 512 : (th + 1) * 512],
                            start=(kc == 0),
                            stop=(kc == 1),
                        )

                if PHASE < 4:
                    continue
                # ---- P = exp(shu + su[j]) ----
                Pt = p_pool.tile([P, T], MMDT)
                nc.scalar.activation(Pt[:], st[:], AF.Exp, bias=su[:], scale=1.0)

                if PHASE < 5:
                    continue
                # ---- shT[t-chunked] = HT-chunk.T @ w_h column (N=2 matmuls;
                # fp32r requires even N, so the w_h column is duplicated) ----
                shT_ps = sm_ps.tile([P, NT, 2], F32, tag="sm")
                for c in range(NT):
                    for kc in range(2):
                        nc.tensor.matmul(
                            shT_ps[:, c, :],
                            HT[:, kc, c * P : (c + 1) * P],
                            whT[:, kc, :],
                            start=(kc == 0),
                            stop=(kc == 1),
                        )
                esh = sm_pool.tile([P, NT], F32)
                nc.scalar.activation(esh[:], shT_ps[:, :, 0], AF.Exp)

                if PHASE < 6:
                    continue
                # ---- max_j P via PE transpose + one DVE reduce ----
                pt = big_ps.tile([P, T], MMDT, tag="big")
                for c in range(NT):
                    nc.tensor.transpose(
                        pt[:, c * P : (c + 1) * P],
                        Pt[:, c * P : (c + 1) * P],
                        ident[:],
                    )
                mx = sm_pool.tile([P, NT], F32)
                nc.vector.reduce_max(
                    mx[:].unsqueeze(2),
                    pt[:].bitcast(F32).rearrange("p (c j) -> p c j", j=P),
                    axis=AX.X,
                )
                wq = sm_pool.tile([P, NT], MMDT)
                nc.vector.tensor_mul(wq[:], mx[:], esh[:])

                if PHASE < 7:
                    continue
                # ---- C2Q = softmax_j(S)-matmul: per t-chunk ----
                C2Q = g_pool.tile([P, NT, D], F32)
                linv = sm_pool.tile([P, NT], F32)
                for c in range(NT):
                    cq = cq_ps.tile([P, 258], F32, tag="cq")
                    nc.tensor.matmul(
                        cq[:],
                        Pt[:, c * P : (c + 1) * P],
                        Uo[:, 0:258],
                        start=True,
                        stop=True,
                    )
                    nc.vector.reciprocal(linv[:, c : c + 1], cq[:, 256:257])
                    if c % 2 == 0:
                        nc.scalar.activation(
                            C2Q[:, c, :],
                            cq[:, 0:256],
                            AF.Copy,
                            scale=linv[:, c : c + 1],
                        )
                    else:
                        nc.vector.tensor_scalar_mul(
                            C2Q[:, c, :], cq[:, 0:256], linv[:, c : c + 1]
                        )
                nc.sync.dma_start(Gb[:, :, 1, :], C2Q[:])

                if PHASE < 8:
                    continue
                # ---- G3 = H * C2Q ----
                G3 = g_pool.tile([P, NT, D], F32)
                nc.vector.tensor_mul(
                    G3[:, 0:4, :], Hn[:, 0:4, 0:D], C2Q[:, 0:4, :]
                )
                nc.gpsimd.tensor_mul(
                    G3[:, 4:8, :], Hn[:, 4:8, 0:D], C2Q[:, 4:8, :]
                )
                nc.sync.dma_start(Gb[:, :, 2, :], G3[:])

                if PHASE < 9:
                    continue
                # ---- Q2C ----
                q2cu_ps = sm_ps.tile([1, 258], F32, tag="sm")
                for c in range(NT):
                    nc.tensor.matmul(
                        q2cu_ps[:],
                        wq[:, c : c + 1],
                        Hnr[:, c, 0:258],
                        start=(c == 0),
                        stop=(c == NT - 1),
                    )
                q2cu = sm_pool.tile([1, 257], F32)
                nc.scalar.copy(q2cu[:], q2cu_ps[:, 0:257])
                rin = sm_pool.tile([1, 1], F32)
                nc.vector.reciprocal(rin[:], q2cu[:, 256:257])
                q2cn = sm_pool.tile([1, D], MMDT)
                nc.scalar.activation(
                    q2cn[:], q2cu[:, 0:256], AF.Copy, scale=rin[:]
                )
                # broadcast Q2C across partitions with a K=1 ones-matmul
                qb_ps = half_ps.tile([P, D], F32, tag="half")
                nc.tensor.matmul(qb_ps[:], ones1[:], q2cn[:], start=True, stop=True)
                q2cb = sm_pool.tile([P, D], F32)
                nc.scalar.copy(q2cb[:], qb_ps[:])

                if PHASE < 10:
                    continue
                # ---- G4 = H * Q2C (free-dim broadcast of q2cb) ----
                G4 = g_pool.tile([P, NT, D], F32)
                nc.gpsimd.tensor_mul(
                    G4[:, 0:4, :],
                    Hn[:, 0:4, 0:D],
                    q2cb[:].unsqueeze(1).broadcast_to((P, 4, D)),
                )
                nc.vector.tensor_mul(
                    G4[:, 4:8, :],
                    Hn[:, 4:8, 0:D],
                    q2cb[:].unsqueeze(1).broadcast_to((P, 4, D)),
                )
                nc.sync.dma_start(Gb[:, :, 3, :], G4[:])

    return nc


_NC_CACHE = {}


def get_nc(bpc=BPC):
    key = (bpc, PHASE)
    if key not in _NC_CACHE:
        import bass_rust as _bass_rust

        nc = bass.Bass()
        build_kernel(nc, bpc)
        # TRN2 allows at most 1 sync wait per instruction (2 on event
        # semaphores); Tile emits more.  These are the bacc lowering passes
        # that legalize the wait lists.
        _bass_rust.move_matmul_waits_to_ldweights(nc.m)
        _bass_rust.generate_event_semaphores(nc)
        # lower bass_isa subclasses (e.g. EVENT_SEMAPHORE_RANGE_CLEAR) into
        # raw InstISA encodings walrus can emit
        mybir.codegen_inst_isa_subclasses(nc)
        _NC_CACHE[key] = nc
    return _NC_CACHE[key]


def _augment(x):
    """[..., D] f32 -> [..., DA] with column D = 1.0 (rest pad 1.0)."""
    out = np.ones(x.shape[:-1] + (DA,), dtype=np.float32)
    out[..., :D] = x
    return out


def run(inputs, trace=False, **kwargs):
    from concourse.bass_utils import run_bass_kernel_spmd

    nc = get_nc(BPC)
    H = _augment(np.asarray(inputs["H"], dtype=np.float32))
    U = _augment(np.asarray(inputs["U"], dtype=np.float32))
    w_h = np.asarray(inputs["w_h"], dtype=np.float32)
    whT = np.ascontiguousarray(
        np.repeat(w_h.reshape(2, P).T[:, :, None], 2, axis=2)
    )
    w2 = np.stack(
        [
            np.asarray(inputs["w_hu"], dtype=np.float32),
            np.asarray(inputs["w_u"], dtype=np.float32),
        ]
    )
    ident = np.eye(P, dtype=np.float32)
    ones1 = np.ones((1, P), dtype=np.float32)
    in_maps = [
        {
            "H": H[c * BPC : (c + 1) * BPC],
            "U": U[c * BPC : (c + 1) * BPC],
            "whT": whT,
            "w2": w2,
            "ident": ident,
            "ones1": ones1,
        }
        for c in range(NCORES)
    ]
    res = run_bass_kernel_spmd(
        nc, in_maps, core_ids=list(range(NCORES)), trace=trace, **kwargs
    )
    out = np.concatenate([res.results[c]["G"] for c in range(NCORES)], axis=0)
    return out, res


def kernel(**inputs):
    out, _ = run(inputs, trace=False)
    return out



# revision 5
# speedup vs baseline: 2.0529x; 2.0529x over previous
"""BIDAF attention-flow kernel for Trainium2 (Bass/Tile), 8-core data-parallel.

Reference computation (per batch b):
    S[t,j]  = H[t]·w_h + U[j]·w_u + sum_d H[t,d]*U[j,d]*w_hu[d]
    A       = softmax_j(S);          C2Q = A @ U
    b_att   = softmax_t(max_j S);    Q2C = b_att @ H   (broadcast over t)
    G       = [H, C2Q, H*C2Q, H*Q2C]        # [T, 4D]

v2 design (per core, 8 batches), all-bf16 matmuls + bf16 output:
  * Identity  sum_d H[t,d]*w_h[d] = sum_d H[t,d]*(w_h[d]*1)  folds w_h into
    the U-side weights:  S[t,j] = sum_d (U[j,d]*w_hu[d] + w_h[d])*H[t,d]
    + su[j].  No separate sh row, no esh: wq[t] = max_j exp(S) directly.
  * Host supplies H in BOTH layouts as bf16 (t-major for Q2C rhs /
    elementwise G blocks, d-major for the similarity rhs), so the kernel does
    zero H transposes and no H SBUF copies.  U likewise (j-major + d-major).
  * UwT (the similarity lhsT, [d,j]) is built from Ut with one ACT op per
    128-half: scale=w_hu (per-partition), bias=w_h (per-partition).
  * ST[j,t] = UwT.T @ Hdt;  P = exp(ST + su[j]) (ACT bias).  C2Q chunk
    matmuls use P chunks as lhsT directly.  The U ones-column yields the
    softmax normalizer l[t]; the H ones-column yields Wsum for Q2C.
  * max_j P: PE re-transposes P (bf16, 1 cyc/row) and one DVE reduce_max.
  * Q2C: lhsT = wq column broadcast to M=128 (stride-0), so the accumulated
    PSUM [128,258] is the Q2C row already broadcast to every partition;
    normalize+cast in the mandatory PSUM->SBUF ACT copy.
  * G is written to DRAM in bf16, [b, p, g, c, d] layout (4KB contiguous per
    partition per block); the host expands to f32 and un-permutes.  bf16
    rounding is ~4e-3 max rel err vs the 2e-2 gate.
  * Tile emits multi-wait instructions; TRN2 allows 1 wait/instruction, so
    the bacc rust passes (move_matmul_waits_to_ldweights +
    generate_event_semaphores) are run on the traced module before compile.
"""

import os
import sys

sys.path.insert(0, "/opt/trn_rl_repo")

import numpy as np
import ml_dtypes

import concourse.bass as bass
import concourse.mybir as mybir
from concourse import tile

B, T, J, D = 64, 1024, 128, 256
NCORES = 8
BPC = B // NCORES  # batches per core
P = 128
NT = T // P  # 8 t-chunks per batch
DA = 260  # augmented feature dim: [x | 1 | pad(1.0)*3]
F32 = mybir.dt.float32
BF = mybir.dt.bfloat16
AF = mybir.ActivationFunctionType
ALU = mybir.AluOpType
AX = mybir.AxisListType

PHASE = int(os.environ.get("KPHASE", "10"))


def build_kernel(nc, bpc):
    Htd = nc.declare_dram_parameter("Htd", [bpc, P, NT, DA], BF, isOutput=False)
    Hdt = nc.declare_dram_parameter("Hdt", [bpc, P, 2, T], BF, isOutput=False)
    Ubp = nc.declare_dram_parameter("Ub", [bpc, P, DA], BF, isOutput=False)
    Utp = nc.declare_dram_parameter("Ut", [bpc, P, 2, P], BF, isOutput=False)
    wcol_in = nc.declare_dram_parameter("wcol", [P, 2, 2], F32, isOutput=False)
    wub_in = nc.declare_dram_parameter("wub", [P, D], BF, isOutput=False)
    ident_in = nc.declare_dram_parameter("identb", [P, P], BF, isOutput=False)
    G = nc.declare_dram_parameter("G", [bpc, P, 4, NT, D], BF, isOutput=True)

    with tile.TileContext(nc) as tc:
        with (
            tc.tile_pool(name="const", bufs=1) as const_pool,
            tc.tile_pool(name="h", bufs=2) as h_pool,
            tc.tile_pool(name="ht", bufs=2) as ht_pool,
            tc.tile_pool(name="p", bufs=2) as p_pool,
            tc.tile_pool(name="g", bufs=2) as g_pool,
            tc.tile_pool(name="u", bufs=2) as u_pool,
            tc.tile_pool(name="sm", bufs=2) as sm_pool,
            tc.tile_pool(name="stps", bufs=1, space="PSUM") as st_ps,
            tc.tile_pool(name="ptps", bufs=2, space="PSUM") as pt_ps,
            tc.tile_pool(name="cqps", bufs=2, space="PSUM") as cq_ps,
            tc.tile_pool(name="qbps", bufs=2, space="PSUM") as qb_ps,
        ):
            # ---- constants ----
            ident = const_pool.tile([P, P], BF)
            nc.sync.dma_start(ident[:], ident_in[:])
            wcol = const_pool.tile([P, 2, 2], F32)
            nc.sync.dma_start(wcol[:], wcol_in[:])
            wub = const_pool.tile([P, D], BF)
            nc.sync.dma_start(wub[:], wub_in[:])

            for b in range(bpc):
                # ---- load inputs ----
                Hn = h_pool.tile([P, NT, DA], BF)
                nc.sync.dma_start(Hn[:], Htd[b])
                Gb = G[b]
                # G block 0 = H (bf16): pure DMA, goes out immediately
                nc.sync.dma_start(Gb[:, 0, :, :], Hn[:, :, 0:D])

                HT = ht_pool.tile([P, 2, T], BF)
                nc.sync.dma_start(HT[:], Hdt[b])
                Ub = u_pool.tile([P, DA], BF)
                nc.sync.dma_start(Ub[:], Ubp[b])
                Ut = u_pool.tile([P, 2, P], BF)
                nc.sync.dma_start(Ut[:], Utp[b])

                if PHASE < 2:
                    continue
                # ---- U-side prep ----
                # UwT[d, j] = Ut[d, j]*w_hu[d] + w_h[d]  (per-partition d)
                UwT = u_pool.tile([P, 2, P], BF)
                for kc in range(2):
                    nc.scalar.activation(
                        UwT[:, kc, :],
                        Ut[:, kc, :],
                        AF.Identity,
                        scale=wcol[:, kc, 0:1],
                        bias=wcol[:, kc, 1:2],
                    )
                # su[j] = U[j]·w_u
                scr = sm_pool.tile([P, D], BF)
                nc.vector.tensor_mul(scr[:], Ub[:, 0:D], wub[:])
                su = sm_pool.tile([P, 1], F32)
                nc.vector.reduce_sum(su[:], scr[:], axis=AX.X)

                if PHASE < 3:
                    continue
                # ---- similarity: st[j, t] = sum_d UwT[d,j]*HT[d,t] ----
                st = st_ps.tile([P, T], F32, tag="st")
                for kc in range(2):
                    for th in range(2):
                        nc.tensor.matmul(
                            st[:, th * 512 : (th + 1) * 512],
                            UwT[:, kc, :],
                            HT[:, kc, th * 512 : (th + 1) * 512],
                            start=(kc == 0),
                            stop=(kc == 1),
                        )

                if PHASE < 4:
                    continue
                # ---- P = exp(st + su[j]) ----
                Pt = p_pool.tile([P, T], BF)
                nc.scalar.activation(Pt[:], st[:], AF.Exp, bias=su[:], scale=1.0)

                if PHASE < 5:
                    continue
                # ---- wq[t] = max_j P via PE transpose + one DVE reduce ----
                ptp = pt_ps.tile([P, T], BF, tag="pt")
                for c in range(NT):
                    nc.tensor.transpose(
                        ptp[:, c * P : (c + 1) * P],
                        Pt[:, c * P : (c + 1) * P],
                        ident[:],
                    )
                wq = sm_pool.tile([P, NT], BF)
                nc.vector.reduce_max(
                    wq[:].unsqueeze(2),
                    ptp[:].rearrange("p (c j) -> p c j", j=P),
                    axis=AX.X,
                )

                if PHASE < 6:
                    continue
                # ---- C2Q = softmax_j(S) @ U, per t-chunk ----
                C2Q = g_pool.tile([P, NT, D], BF)
                linv = sm_pool.tile([P, NT], F32)
                for c in range(NT):
                    cq = cq_ps.tile([P, 258], F32, tag="cq")
                    nc.tensor.matmul(
                        cq[:],
                        Pt[:, c * P : (c + 1) * P],
                        Ub[:, 0:258],
                        start=True,
                        stop=True,
                    )
                    nc.vector.reciprocal(linv[:, c : c + 1], cq[:, 256:257])
                    if c % 2 == 0:
                        nc.scalar.activation(
                            C2Q[:, c, :],
                            cq[:, 0:D],
                            AF.Copy,
                            scale=linv[:, c : c + 1],
                        )
                    else:
                        nc.vector.tensor_scalar_mul(
                            C2Q[:, c, :], cq[:, 0:D], linv[:, c : c + 1]
                        )
                nc.sync.dma_start(Gb[:, 1, :, :], C2Q[:])

                if PHASE < 7:
                    continue
                # ---- G2 = H * C2Q ----
                G2 = g_pool.tile([P, NT, D], BF)
                nc.vector.tensor_mul(G2[:], Hn[:, :, 0:D], C2Q[:])
                nc.sync.dma_start(Gb[:, 2, :, :], G2[:])

                if PHASE < 8:
                    continue
                # ---- Q2C, pre-broadcast: qb[p, d] = sum_t wq[t]*H[t, d] ----
                qb = qb_ps.tile([P, 258], F32, tag="qb")
                for c in range(NT):
                    nc.tensor.matmul(
                        qb[:],
                        wq[:, c : c + 1].broadcast_to((P, P)),
                        Hn[:, c, 0:258],
                        start=(c == 0),
                        stop=(c == NT - 1),
                    )
                rin = sm_pool.tile([P, 1], F32)
                nc.vector.reciprocal(rin[:], qb[:, 256:257])
                q2cb = sm_pool.tile([P, D], BF)
                nc.scalar.activation(q2cb[:], qb[:, 0:D], AF.Copy, scale=rin[:])

                if PHASE < 9:
                    continue
                # ---- G3 = H * Q2C (free-dim broadcast of q2cb) ----
                G4 = g_pool.tile([P, NT, D], BF)
                nc.vector.tensor_mul(
                    G4[:],
                    Hn[:, :, 0:D],
                    q2cb[:].unsqueeze(1).broadcast_to((P, NT, D)),
                )
                nc.sync.dma_start(Gb[:, 3, :, :], G4[:])

    return nc


_NC_CACHE = {}


def get_nc(bpc=BPC):
    key = (bpc, PHASE)
    if key not in _NC_CACHE:
        import bass_rust as _bass_rust

        nc = bass.Bass()
        build_kernel(nc, bpc)
        # TRN2 allows at most 1 sync wait per instruction (2 on event
        # semaphores); Tile emits more.  These are the bacc lowering passes
        # that legalize the wait lists.
        _bass_rust.move_matmul_waits_to_ldweights(nc.m)
        _bass_rust.generate_event_semaphores(nc)
        # lower bass_isa subclasses (e.g. EVENT_SEMAPHORE_RANGE_CLEAR) into
        # raw InstISA encodings walrus can emit
        mybir.codegen_inst_isa_subclasses(nc)
        _NC_CACHE[key] = nc
    return _NC_CACHE[key]


def _prep_core(Hc, Uc, w_h, w_hu):
    """Host-side layout prep for one core's batches (all bf16)."""
    bpc = Hc.shape[0]
    # Htd[b, p, c, d]: H[b, c*128+p, d], col 256 = 1.0, pad 1.0
    Htd = np.ones((bpc, NT, P, DA), dtype=ml_dtypes.bfloat16)
    Htd[:, :, :, :D] = Hc.reshape(bpc, NT, P, D).astype(ml_dtypes.bfloat16)
    Htd = np.ascontiguousarray(Htd.transpose(0, 2, 1, 3))
    # Hdt[b, pd, kc, t] = H[b, t, kc*128+pd]
    Hdt = np.ascontiguousarray(
        Hc.astype(ml_dtypes.bfloat16)
        .transpose(0, 2, 1)
        .reshape(bpc, 2, P, T)
        .transpose(0, 2, 1, 3)
    )
    # Ub[b, j, d] with ones column
    Ub = np.ones((bpc, P, DA), dtype=ml_dtypes.bfloat16)
    Ub[:, :, :D] = Uc.astype(ml_dtypes.bfloat16)
    # Ut[b, pd, kc, j] = U[b, j, kc*128+pd]
    Ut = np.ascontiguousarray(
        Uc.astype(ml_dtypes.bfloat16)
        .transpose(0, 2, 1)
        .reshape(bpc, 2, P, P)
        .transpose(0, 2, 1, 3)
    )
    return Htd, Hdt, Ub, Ut


def run(inputs, trace=False, **kwargs):
    from concourse.bass_utils import run_bass_kernel_spmd

    nc = get_nc(BPC)
    H = np.asarray(inputs["H"], dtype=np.float32)
    U = np.asarray(inputs["U"], dtype=np.float32)
    w_h = np.asarray(inputs["w_h"], dtype=np.float32)
    w_u = np.asarray(inputs["w_u"], dtype=np.float32)
    w_hu = np.asarray(inputs["w_hu"], dtype=np.float32)
    # wcol[p, kc, 0] = w_hu[kc*128+p] (ACT scale), wcol[p, kc, 1] = w_h (bias)
    wcol = np.stack(
        [w_hu.reshape(2, P).T, w_h.reshape(2, P).T], axis=2
    ).astype(np.float32)
    wcol = np.ascontiguousarray(wcol)
    wub = np.broadcast_to(w_u.astype(ml_dtypes.bfloat16), (P, D)).copy()
    identb = np.eye(P, dtype=ml_dtypes.bfloat16)

    in_maps = []
    for c in range(NCORES):
        Hc = H[c * BPC : (c + 1) * BPC]
        Uc = U[c * BPC : (c + 1) * BPC]
        Htd, Hdt, Ub, Ut = _prep_core(Hc, Uc, w_h, w_hu)
        in_maps.append(
            {
                "Htd": Htd,
                "Hdt": Hdt,
                "Ub": Ub,
                "Ut": Ut,
                "wcol": wcol,
                "wub": wub,
                "identb": identb,
            }
        )
    res = run_bass_kernel_spmd(
        nc, in_maps, core_ids=list(range(NCORES)), trace=trace, **kwargs
    )
    # G_dev[b, p, g, c, d] -> out[b, c*128+p, g*256+d]
    outs = []
    for c in range(NCORES):
        g = np.asarray(res.results[c]["G"]).astype(np.float32)
        outs.append(
            np.ascontiguousarray(g.transpose(0, 3, 1, 2, 4)).reshape(
                BPC, T, 4 * D
            )
        )
    out = np.concatenate(outs, axis=0)
    return out, res


def kernel(**inputs):
    out, _ = run(inputs, trace=False)
    return out
